# revision 1
# baseline (speedup 1.0000x reference)
"""Bahdanau-attention scores kernel for Trainium2 (8 NeuronCores, SPMD).

Computation (per batch row b):
    pre[s, k] = hidden[b] @ Wh + enc[b, s] @ We + b_attn       (S=1024, E=K=1024)
    scores[s] = tanh(pre[s, :]) @ v
    out[b]    = softmax(where(mask[b]==0, -1e10, scores))      over s

Sharding: data-parallel over batch B=64 -> 8 batches per core; weights
replicated. No collectives.

Per-core structure (fp8 DoubleRow main matmul, bf16 elsewhere):
  - Weights are pre-packed on the host: Wh/hiddenT/b_attn/v in bf16,
    We scaled x64 into E4M3 with the DoubleRow row-pair layout
    w8[p, et, j, k] = 64 * We[et*256 + 2p + j, k]. Loaded on the ACT
    HWDGE ring so they never block the enc-transpose (SP) ring.
  - enc[b] is cast f32->fp8 by DRAM->DRAM SWDGE DMAs into a bounce buffer,
    then one DRAM->SBUF xbar transpose per batch moves the fp8 pairs as
    uint16: encT8[p, et', s] as u16 holds
    (enc[s, et'*256+2p], enc[s, et'*256+2p+1]) -- exactly the DoubleRow
    rhs pairing, so each 256-deep contraction needs one MM:
      pre[k, s] = sum_et' lhsT(w8) @ rhs(encT8)   [4 MMs per (kt, sb)]
    (An SBUF->SBUF variant of the transpose loses badly: Tile serializes
    xbar transposes against in-flight SBUF-writing DMAs -- deadlock guard.)
  - ScalarE: tanh(psum/64 + (hidden@Wh + b_attn)[k]) -> SBUF bf16 (scale
    fuses the x64 We quantization scale)
  - v-dot: 4 col-tiled PE matmuls (tile_position=(0,32q), one s-quarter of
    256 each) run concurrently in separate col-groups of the array, each
    accumulating over k-tiles into PSUM row 32q -- 4x fewer PE cycles than
    one full-width matmul per s-half. DVE copies row 32q to flat4[32q].
  - 4 SWDGE gathers turn flat4 rows into [BL, S]; masked softmax on DVE/ACT.

Sync note: this walrus build encodes at most ONE semaphore wait per
instruction; _split_multi_waits() rewrites Tile's multi-wait instructions
into NoOp(wait) chains on the same engine.
"""

import sys

if "/opt/trn_rl_repo" not in sys.path:
    sys.path.insert(0, "/opt/trn_rl_repo")

from contextlib import ExitStack

import numpy as np

B, S, E, K = 64, 1024, 1024, 1024  # E = 2*ENC_HID, K = DEC_HID
NCORES = 8
BL = B // NCORES  # batches per core
NEG = -1e10
WSCALE = 64.0     # We quantization scale into E4M3 range

ET2 = E // 256  # 4 DoubleRow e-tiles (256-deep contraction each)
KT = K // 128   # 8 k-tiles
ST = S // 128   # 8 s-tiles
NB = 512        # matmul free-dim block
SB = S // NB    # 2 s-blocks

_CACHE = {}


def _build_bass(strip=True):
    from concourse import bass, mybir, tile

    f32 = mybir.dt.float32
    bf16 = mybir.dt.bfloat16
    f8 = mybir.dt.float8e4
    u16 = mybir.dt.uint16
    i32 = mybir.dt.int32
    Tanh = mybir.ActivationFunctionType.Tanh
    Exp = mybir.ActivationFunctionType.Exp
    Alu = mybir.AluOpType
    Ax = mybir.AxisListType
    DR = mybir.MatmulPerfMode.DoubleRow

    nc = bass.Bass()

    enc_d = nc.declare_dram_parameter("encoder_outputs", [BL, S, E], f32, isOutput=False)
    enc8_d = nc.dram_tensor("enc8", [BL, S, E], f8)
    mask_d = nc.declare_dram_parameter("mask", [BL, S], i32, isOutput=False)
    wh_d = nc.declare_dram_parameter("wh_pack", [128, KT, K], bf16, isOutput=False)
    w8_d = nc.declare_dram_parameter("w8_pack", [128, ET2, 2, K], f8, isOutput=False)
    hT_d = nc.declare_dram_parameter("hT_pack", [128, KT * BL], bf16, isOutput=False)
    b_d = nc.declare_dram_parameter("b_pack", [1, K], bf16, isOutput=False)
    v_d = nc.declare_dram_parameter("v_pack", [128, KT], bf16, isOutput=False)
    out_d = nc.declare_dram_parameter("out", [BL, S], f32, isOutput=True)

    with tile.TileContext(nc) as tc, ExitStack() as ctx:
        const = ctx.enter_context(tc.tile_pool(name="const", bufs=1))
        tp_pool = ctx.enter_context(tc.tile_pool(name="encT", bufs=4))
        tanh_pool = ctx.enter_context(tc.tile_pool(name="tanh", bufs=4))
        pre_ps = ctx.enter_context(tc.tile_pool(name="pre_ps", bufs=3, space="PSUM"))
        sc_ps = ctx.enter_context(tc.tile_pool(name="sc_ps", bufs=1, space="PSUM"))
        fin = ctx.enter_context(tc.tile_pool(name="fin", bufs=1))

        # ---- weights (host-packed) ----
        # w8 gates the first matmul: ACT HWDGE ring, first in line. Wh
        # follows it there -- hp's matmuls sit at the PE FIFO head, so Wh
        # arriving late would block every main matmul behind them.
        w8 = const.tile([128, ET2, 2, K], f8)
        nc.scalar.dma_start(w8[:], w8_d[:])
        # small tensors on the SP ring (fast to dispatch, then the ring is
        # all transposes)
        hT_bf = const.tile([128, KT * BL], bf16)
        nc.sync.dma_start(hT_bf[:], hT_d[:])
        b_attn_bf = const.tile([1, K], bf16)
        nc.sync.dma_start(b_attn_bf[:], b_d[:])
        v_bf = const.tile([128, KT], bf16)
        nc.sync.dma_start(v_bf[:], v_d[:])
        wh_bf = const.tile([128, KT, K], bf16)

        ones_bf = const.tile([1, BL], bf16)
        nc.vector.memset(ones_bf[:], 1.0)

        def stage_tp(b):
            """One DRAM->SBUF u16-pair xbar transpose of the fp8 bounce:
            encT8 u16[p, et', s] = fp8 pair
            (enc[b, s, et'*256+2p], enc[b, s, et'*256+2p+1])."""
            encT8 = tp_pool.tile([128, ET2, S], u16, tag="encT", name="encT8")
            nc.sync.dma_start(
                encT8[:], enc8_d[b].bitcast(u16), transpose=True)
            return encT8

        def stage_cast(b):
            for st in range(ST):
                nc.gpsimd.dma_start(
                    enc8_d[b, st * 128:(st + 1) * 128, :],
                    enc_d[b, st * 128:(st + 1) * 128, :])

        hpb = const.tile([128, KT * BL], f32)  # col = kt*BL + b

        def emit_hp():
            # h_proj[k, b] = sum_d Wh[d, k]*hidden[b, d] + b_attn[k]
            hp_ps = pre_ps.tile([128, NB], f32, tag="pre", name="hp_ps",
                                bufs=7)
            for kt in range(KT):
                for dt in range(KT):
                    nc.tensor.matmul(
                        hp_ps[:, kt * BL:(kt + 1) * BL],
                        wh_bf[:, dt, kt * 128:(kt + 1) * 128],
                        hT_bf[:, dt * BL:(dt + 1) * BL],
                        start=(dt == 0),
                        stop=False,
                    )
                nc.tensor.matmul(
                    hp_ps[:, kt * BL:(kt + 1) * BL],
                    b_attn_bf[:, kt * 128:(kt + 1) * 128],
                    ones_bf[:],
                    start=False,
                    stop=True,
                )
            nc.vector.tensor_copy(hpb[:], hp_ps[:, :KT * BL])

        # prologue: b0's cast in 4 quarter-chunks (minimal Q7 issue time,
        # the first transpose only needs the first two)
        for q in range(4):
            nc.gpsimd.dma_start(
                enc8_d[0, q * 256:(q + 1) * 256, :],
                enc_d[0, q * 256:(q + 1) * 256, :])
        nc.scalar.dma_start(wh_bf[:, :, :NB], wh_d[:, :, :NB])
        nc.scalar.dma_start(wh_bf[:, :, NB:], wh_d[:, :, NB:])
        for b in range(1, 3):
            stage_cast(b)
        # b0's transpose split into two separate half tiles so compute can
        # start after only half the casts; b0's compute loop is sb-major
        encT0h = []
        for h in range(SB):
            eh = tp_pool.tile([128, ET2, NB], u16, tag=f"encT0h{h}",
                              name=f"encT0h{h}")
            nc.sync.dma_start(
                eh[:],
                enc8_d[0, h * NB:(h + 1) * NB, :].bitcast(u16),
                transpose=True)
            encT0h.append(eh)
        encTs = {1: stage_tp(1)}

        emit_hp()

        # scores accumulate on PSUM rows 32q (col-group q = s-quarter q);
        # staged in flat4 rows 32q on SBUF, gathered to [4, S] halves at the
        # end
        flat4 = fin.tile([97, BL * 256], f32)
        HB = BL // 2

        # softmax state. The scores tile is pre-filled with the mask offset
        # (mask-1)*1e10 mid-kernel; the end-gathers then ACCUMULATE flat4
        # into it (SWDGE accum_op=add), so the masked add costs nothing.
        mask_i = fin.tile([BL, S], i32)
        mask_f = fin.tile([BL, S], f32)
        scores = fin.tile([BL, S], f32)
        negmax = fin.tile([BL, 1], f32)
        expv = fin.tile([BL, S], f32)
        rowsum = fin.tile([BL, 1], f32)
        recip = fin.tile([BL, 1], f32)
        outf = fin.tile([BL, S], f32)

        def emit_mask_prep():
            # mask is a tail-only input: load mid-kernel on the ACT ring
            nc.scalar.dma_start(mask_i[:], mask_d[:])
            nc.vector.tensor_copy(mask_f[:], mask_i[:])
            nc.vector.tensor_scalar(
                scores[:], mask_f[:], -NEG, NEG, Alu.mult, Alu.add)

        def emit_softmax():
            # Each gather reads a FULL flat4 row (all 8 batches): the b7
            # dependency keeps these SBUF->SBUF DMAs from being scheduled
            # while xbar transposes are still in flight (Tile serializes
            # that pair -- HW deadlock guard -- which would starve the PE).
            for q in range(4):
                nc.gpsimd.dma_start(
                    scores[:, q * 256:(q + 1) * 256],
                    flat4[32 * q:32 * q + 1, :],
                    accum_op=Alu.add)
            # adding (mask-1)*1e10 alone is enough: exp(score-1e10-max) == 0
            nc.vector.tensor_reduce(
                negmax[:], scores[:], Ax.X, Alu.max, negate=True)
            nc.scalar.activation(
                expv[:], scores[:], Exp, bias=negmax[:],
                accum_out=rowsum[:])
            nc.vector.reciprocal(recip[:], rowsum[:])
            nc.vector.tensor_scalar_mul(outf[:], expv[:], recip[:])
            nc.scalar.dma_start(out_d[:], outf[:])

        # ---- main loop over local batches (software-pipelined, 2 deep) ----
        # carry: v-dots for the last two k-tiles of batch b-1 are emitted
        # after batch b's first main-MM groups, so the PE never idles at the
        # batch boundary waiting for the previous batch's trailing tanh.
        # scores PSUM: ONE bank, halves alternated by batch parity (PSUM is
        # fully booked: 7 pre banks + this)
        scband = sc_ps.tile([128, 2, 256], f32, tag="sc", name="scband")

        def scq(b, q):
            return scband[32 * q:32 * q + 1, b % 2, :]

        carry = []  # [(b_prev, kt, th, qs)] not yet emitted

        def emit_carry(budget):
            while carry and budget > 0:
                b_p, kt, th, qs = carry.pop(0)
                for q in qs:
                    off = (q - qs[0]) * 256 if len(qs) < 4 else q * 256
                    nc.tensor.matmul(
                        scq(b_p, q),
                        v_bf[:, kt:kt + 1],
                        th[:, off:off + 256],
                        start=False, stop=(kt == KT - 1),
                        tile_position=(0, 32 * q))
                    if kt == KT - 1:
                        nc.vector.tensor_copy(
                            flat4[32 * q:32 * q + 1,
                                  b_p * 256:(b_p + 1) * 256],
                            scq(b_p, q))
                budget -= 1

        for b in range(BL):
            # cast lookahead deferred one iter: at iter 0 the DMA system is
            # already saturated with weights + b0..b2 casts + the first
            # transposes; issuing cast(3) there halves tp(2)'s bandwidth and
            # stalls the PE ~12us at b2 (plus a HAM re-throttle)
            if b >= 1 and b + 2 < BL:
                stage_cast(b + 2)
            if b + 2 < BL:
                encTs[b + 2] = stage_tp(b + 2)
            if b > 0:
                encT8 = encTs.pop(b)
                # rhs view: [p, et', j, s]; j = fp8 pair index inside u16
                rhsv = encT8[:].bitcast(f8).rearrange(
                    "p et (s j) -> p et j s", j=2)
            if b == 0:
                # sb-major: start on the first transposed s-half immediately
                for sb in range(SB):
                    rh = encT0h[sb][:].bitcast(f8).rearrange(
                        "p et (s j) -> p et j s", j=2)
                    ths = {}
                    for kt in range(KT):
                        pre = pre_ps.tile([128, NB], f32, tag="pre",
                                          name="preh", bufs=7)
                        for et in range(ET2):
                            nc.tensor.matmul(
                                pre[:],
                                w8[:, et, :, kt * 128:(kt + 1) * 128],
                                rh[:, et, :, :],
                                start=(et == 0),
                                stop=(et == ET2 - 1),
                                perf_mode=DR,
                            )
                        if kt == 0:
                            emit_carry(2)
                        th = tanh_pool.tile([128, NB], bf16, tag="thh",
                                            name="thh", bufs=5)
                        nc.scalar.activation(
                            th[:], pre[:], Tanh,
                            bias=hpb[:, kt * BL:kt * BL + 1],
                            scale=1.0 / WSCALE,
                        )
                        ths[kt] = th
                        if kt > 0:
                            for h in range(2):
                                q = 2 * sb + h
                                nc.tensor.matmul(
                                    scq(0, q),
                                    v_bf[:, kt - 1:kt],
                                    ths[kt - 1][:, h * 256:(h + 1) * 256],
                                    start=(kt - 1 == 0), stop=False,
                                    tile_position=(0, 32 * q))
                    carry.append((0, KT - 1, ths[KT - 1],
                                  (2 * sb, 2 * sb + 1)))
            else:
                ths = {}
                for kt in range(KT):
                    # per-sb pre tiles, 7 deep: the kt0 start=True write's
                    # WAR reaches 3.5 k-tiles back, so the PE never waits
                    # for the previous batch's trailing tanh
                    pres = [pre_ps.tile([128, NB], f32, tag="pre",
                                        name=f"pre{sb}", bufs=7)
                            for sb in range(SB)]
                    for et in range(ET2):  # one LDWEIGHTS serves both sb
                        for sb in range(SB):
                            nc.tensor.matmul(
                                pres[sb][:],
                                w8[:, et, :, kt * 128:(kt + 1) * 128],
                                rhsv[:, et, :, sb * NB:(sb + 1) * NB],
                                start=(et == 0),
                                stop=(et == ET2 - 1),
                                perf_mode=DR,
                            )
                    if kt == 0:
                        emit_carry(2)
                    th = tanh_pool.tile([128, SB * NB], bf16, tag="tanh",
                                        bufs=5)
                    for sb in range(SB):
                        nc.scalar.activation(
                            th[:, sb * NB:(sb + 1) * NB], pres[sb][:], Tanh,
                            bias=hpb[:, kt * BL + b:kt * BL + b + 1],
                            scale=1.0 / WSCALE,
                        )
                    ths[kt] = th
                    if kt > 1:
                        for q in range(4):
                            nc.tensor.matmul(
                                scq(b, q),
                                v_bf[:, kt - 2:kt - 1],
                                ths[kt - 2][:, q * 256:(q + 1) * 256],
                                start=(kt - 2 == 0), stop=False,
                                tile_position=(0, 32 * q))
                for q in range(4):
                    nc.tensor.matmul(
                        scq(b, q),
                        v_bf[:, KT - 2:KT - 1],
                        ths[KT - 2][:, q * 256:(q + 1) * 256],
                        start=False, stop=False,
                        tile_position=(0, 32 * q))
                carry.append((b, KT - 1, ths[KT - 1], (0, 1, 2, 3)))
            if b == HB:
                emit_mask_prep()

        emit_carry(len(carry))
        emit_softmax()

    if strip:
        _split_multi_waits(nc, mybir)
    return nc


def _split_multi_waits(nc, mybir):
    """Move extra semaphore waits onto standalone NoOps on the same engine.

    This walrus build encodes at most one sync-wait command per instruction,
    but Tile emits instructions with several (cross-engine RAW + WAR + DMA
    queue ordering). A NoOp carrying one wait, placed immediately before the
    instruction in the same engine's stream, is semantically identical: the
    engine's sequencer blocks on the NoOp's wait before dispatching the real
    instruction.
    """
    n = 0
    for fn in nc.m.functions:
        for blk in fn.blocks:
            insts = blk.instructions
            new = []
            changed = False
            for inst in insts:
                si = inst.sync_info
                if si is not None and si.on_wait and len(si.on_wait) > 1:
                    for w in list(si.on_wait)[:-1]:
                        n += 1
                        new.append(mybir.InstNoOp(
                            name=f"{inst.name}-sw{n}",
                            engine=inst.engine,
                            text_hint="split_wait",
                            bass_nofuse=True,
                            sync_info=mybir.SyncInfo(
                                on_wait=[w], on_update=[]),
                        ))
                    inst.sync_info = mybir.SyncInfo(
                        on_wait=[list(si.on_wait)[-1]],
                        on_update=list(si.on_update or []))
                    changed = True
                new.append(inst)
            if changed:
                blk.instructions = new


def get_nc(strip=True):
    key = ("nc", strip)
    if key not in _CACHE:
        _CACHE[key] = _build_bass(strip)
    return _CACHE[key]


def make_in_maps(hidden, encoder_outputs, mask, W_attn, b_attn, v):
    import ml_dtypes

    bf16 = ml_dtypes.bfloat16
    f8 = ml_dtypes.float8_e4m3

    W_attn = np.asarray(W_attn, dtype=np.float32)
    Wh, We = W_attn[:K], W_attn[K:]
    # wh_pack[p, dt, k] = Wh[dt*128 + p, k]
    wh_pack = np.ascontiguousarray(
        Wh.reshape(KT, 128, K).transpose(1, 0, 2).astype(bf16))
    # w8_pack[p, et, j, k] = 64 * We[et*256 + 2p + j, k]
    w8_pack = np.ascontiguousarray(
        (We * WSCALE).reshape(ET2, 128, 2, K).transpose(1, 0, 2, 3).astype(f8))
    b_pack = np.ascontiguousarray(
        np.asarray(b_attn, dtype=np.float32).reshape(1, K).astype(bf16))
    # v_pack[p, kt] = v[kt*128 + p]
    v_pack = np.ascontiguousarray(
        np.asarray(v, dtype=np.float32).reshape(KT, 128).T.astype(bf16))
    hidden = np.asarray(hidden, dtype=np.float32)

    in_maps = []
    for c in range(NCORES):
        sl = slice(c * BL, (c + 1) * BL)
        # hT_pack[p, dt*BL + b] = hidden[b, dt*128 + p]
        hT_pack = np.ascontiguousarray(
            hidden[sl].T.reshape(KT, 128, BL).transpose(1, 0, 2)
            .reshape(128, KT * BL).astype(bf16))
        in_maps.append({
            "encoder_outputs": np.ascontiguousarray(encoder_outputs[sl]),
            "mask": np.ascontiguousarray(np.asarray(mask[sl], dtype=np.int32)),
            "wh_pack": wh_pack,
            "w8_pack": w8_pack,
            "hT_pack": hT_pack,
            "b_pack": b_pack,
            "v_pack": v_pack,
        })
    return in_maps


def kernel(hidden, encoder_outputs, mask, W_attn, b_attn, v):
    from concourse.bass_utils import run_bass_kernel_spmd

    nc = get_nc()
    in_maps = make_in_maps(hidden, encoder_outputs, mask, W_attn, b_attn, v)
    res = run_bass_kernel_spmd(nc, in_maps, core_ids=list(range(NCORES)))
    return np.concatenate(
        [np.asarray(res.results[c]["out"], dtype=np.float32) for c in range(NCORES)],
        axis=0,
    )

